# revision 23
# baseline (speedup 1.0000x reference)
"""CoAttentionFusion Trainium2 kernel.

Sharding: data-parallel over batch. B=8 -> 1 batch element per NeuronCore.
Each core computes both gated cross-attention blocks + both FFNs for its
batch element; no collectives.

Layout strategy per core:
  - Inputs loaded token-major, LN stats computed token-major (bn_stats),
    normalized tiles PE-transposed once into feature-major (xT) form.
  - All projections consume feature-major activations; LN gains/biases are
    folded into the following weight matrices on the host.
  - Scores are computed twice (s token-major for the softmax/w output; sT
    for the ctx matmul) which is cheaper than transposing w.
  - qp (gate projection of the *raw* input) is reconstructed from the
    normalized input via the identity qf = qn/rstd + m, so the raw input
    never needs a feature-major copy.
"""

import numpy as np

P = 128
EPS = 1e-5
H = 8
E = 512
NCORES = 8

_CACHE = {}


def _f32(x):
    return np.ascontiguousarray(x, dtype=np.float32)


def _pack_w(w):
    # [K*128, E] -> [128, K, E] bf16 (partition-major chunks)
    import ml_dtypes

    K = w.shape[0] // P
    return np.ascontiguousarray(
        w.reshape(K, P, w.shape[1]).transpose(1, 0, 2).astype(ml_dtypes.bfloat16)
    )


def _pack_col(b):
    # [E] -> [128, E//128] f32 (chunk c in column c)
    return np.ascontiguousarray(b.reshape(-1, P).T.astype(np.float32))


def _pack_row(b):
    return np.ascontiguousarray(b.reshape(1, -1).astype(np.float32))


def _prep_inputs_host(inp):
    """Fold LN gains into weights, pack layouts. Returns dict of per-core
    constant arrays (weights) — batch slices added separately."""
    d = {}
    for p, qd, kd in (("t2v", 768, 1024), ("v2t", 1024, 768)):
        g = inp  # shorthand
        nq_g, nq_b = _f32(g[p + "_nq_g"]), _f32(g[p + "_nq_b"])
        nk_g, nk_b = _f32(g[p + "_nk_g"]), _f32(g[p + "_nk_b"])
        wq, bq = _f32(g[p + "_wq"]), _f32(g[p + "_bq"])
        wk, bk = _f32(g[p + "_wk"]), _f32(g[p + "_bk"])
        wv, bv = _f32(g[p + "_wv"]), _f32(g[p + "_bv"])
        wo, bo = _f32(g[p + "_wo"]), _f32(g[p + "_bo"])
        wgp, bgp = _f32(g[p + "_wgp"]), _f32(g[p + "_bgp"])
        g1w, g1b = _f32(g[p + "_g1w"]), _f32(g[p + "_g1b"])
        g2w, g2b = _f32(g[p + "_g2w"]), _f32(g[p + "_g2b"])
        # fold LN affine into q/k/v weights: (core*g + b) @ W = core @ (g[:,None]*W) + b@W
        wq_e = nq_g[:, None] * wq
        bq_e = bq + nq_b @ wq
        wk_e = nk_g[:, None] * wk
        bk_e = bk + nk_b @ wk
        wv_e = nk_g[:, None] * wv
        bv_e = bv + nk_b @ wv
        d[p + "_wq"] = _pack_w(wq_e)
        d[p + "_wk"] = _pack_w(wk_e)
        d[p + "_wv"] = _pack_w(wv_e)
        d[p + "_wo"] = _pack_w(wo)
        d[p + "_wgp"] = _pack_w(wgp)
        d[p + "_g1w"] = _pack_w(g1w)
        d[p + "_g2w"] = _pack_w(g2w)
        d[p + "_bq_c"] = _pack_col(bq_e)
        d[p + "_bk_c"] = _pack_col(bk_e)
        d[p + "_bo_c"] = _pack_col(bo)
        d[p + "_bv_r"] = _pack_row(bv_e)
        d[p + "_bo_r"] = _pack_row(bo)
        d[p + "_bgp_r"] = _pack_row(bgp)
        d[p + "_cgp_r"] = _pack_row(wgp.sum(axis=0))
        d[p + "_g1b_r"] = _pack_row(g1b)
        d[p + "_g2b_r"] = _pack_row(g2b)
        d[p + "_glng"] = _f32(g[p + "_glng"])
        d[p + "_glnb"] = _f32(g[p + "_glnb"])
    for p in ("vff", "tff"):
        lng, lnb = _f32(inp[p + "_lng"]), _f32(inp[p + "_lnb"])
        w1, b1 = _f32(inp[p + "_w1"]), _f32(inp[p + "_b1"])
        w2, b2 = _f32(inp[p + "_w2"]), _f32(inp[p + "_b2"])
        w1_e = lng[:, None] * w1
        b1_e = b1 + lnb @ w1
        d[p + "_w1"] = _pack_w(w1_e)
        d[p + "_w2"] = _pack_w(w2)
        d[p + "_b1_c"] = _pack_col(b1_e)
        d[p + "_b2_r"] = _pack_row(b2)
    e2 = np.zeros((2, P), np.float32)
    e2[0, :64] = 1.0
    e2[1, 64:] = 1.0
    d["e2_const"] = e2
    return d


def _bcast_ap(bass, h, p=P):
    # DRAM AP broadcast across p partitions (stride-0 partition dim)
    ap = h[:]
    return bass.AP(tensor=ap.tensor, offset=ap.offset, ap=[[0, p]] + list(ap.ap))


def _build_program():
    import concourse.bass as bass
    import concourse.tile as tile
    from concourse import masks, mybir
    from contextlib import ExitStack

    f32 = mybir.dt.float32
    bf16 = mybir.dt.bfloat16
    AF = mybir.ActivationFunctionType

    nc = bass.Bass()

    # ---- DRAM parameters (per core) ----
    vis = nc.declare_dram_parameter("vis", [1024, 1024], f32, isOutput=False)
    txt = nc.declare_dram_parameter("txt", [512, 768], f32, isOutput=False)
    prm = {}

    def par(name, shape, dt):
        prm[name] = nc.declare_dram_parameter(name, list(shape), dt, isOutput=False)

    for p, qd, kd in (("t2v", 768, 1024), ("v2t", 1024, 768)):
        Kq, Kk = qd // P, kd // P
        par(p + "_wq", (P, Kq, E), bf16)
        par(p + "_wk", (P, Kk, E), bf16)
        par(p + "_wv", (P, Kk, E), bf16)
        par(p + "_wo", (P, 4, E), bf16)
        par(p + "_wgp", (P, Kq, E), bf16)
        par(p + "_g1w", (P, 8, E), bf16)
        par(p + "_g2w", (P, 4, E), bf16)
        par(p + "_bq_c", (P, 4), f32)
        par(p + "_bk_c", (P, 4), f32)
        par(p + "_bo_c", (P, 4), f32)
        for r in ("_bv_r", "_bo_r", "_bgp_r", "_cgp_r", "_g1b_r", "_g2b_r"):
            par(p + r, (1, E), f32)
        par(p + "_glng", (E,), f32)
        par(p + "_glnb", (E,), f32)
    par("e2_const", (2, P), f32)
    for p, d4 in (("vff", 2048), ("tff", 2048)):
        par(p + "_w1", (P, 4, d4), bf16)
        par(p + "_w2", (P, 16, E), bf16)
        par(p + "_b1_c", (P, 16), f32)
        par(p + "_b2_r", (1, E), f32)

    t2v_w = nc.declare_dram_parameter("t2v_w", [H, 512, 1024], f32, isOutput=True)
    v2t_w = nc.declare_dram_parameter("v2t_w", [H, 1024, 512], f32, isOutput=True)
    vis_out = nc.declare_dram_parameter("vision_out", [1024, E], f32, isOutput=True)
    txt_out = nc.declare_dram_parameter("text_out", [512, E], f32, isOutput=True)

    with ExitStack() as top:
        tc = top.enter_context(tile.TileContext(nc))
        glob = top.enter_context(tc.tile_pool(name="glob", bufs=1))
        persist = top.enter_context(tc.tile_pool(name="persist", bufs=1))

        ident_bf = glob.tile([P, P], bf16, tag="ident_bf")
        masks.make_identity(nc, ident_bf[:])
        ident_f = glob.tile([P, P], f32, tag="ident_f")
        masks.make_identity(nc, ident_f[:])
        ones_row = glob.tile([1, P], f32, tag="ones_row")
        nc.vector.memset(ones_row[:], 1.0)
        e2 = glob.tile([2, P], f32, tag="e2")
        nc.sync.dma_start(out=e2[:], in_=prm["e2_const"][:])
        eps_t = glob.tile([P, 1], f32, tag="eps")
        nc.vector.memset(eps_t[:], EPS)

        # ---------------- Phase A: load + LN + transpose inputs ----------
        import math as _math

        def prep(x_dram, S, D, name):
            nsq, nk = S // P, D // P
            xT = persist.tile([P, nk, S], bf16, tag=name + "_xT")
            m_all = persist.tile([P, nsq], f32, tag=name + "_m")
            rstd_all = persist.tile([P, nsq], f32, tag=name + "_rstd")
            inv_all = persist.tile([P, nsq], f32, tag=name + "_inv")
            rstd_row = persist.tile([1, S], f32, tag=name + "_rstd_row")
            mr_row = persist.tile([1, S], f32, tag=name + "_mr_row")
            fmax = _math.gcd(512, D)
            nsub = D // fmax
            with tc.tile_pool(name=name + "_prep", bufs=3) as pl, tc.tile_pool(
                name=name + "_ptp", bufs=3, space="PSUM"
            ) as ptp:
                for c in range(nsq):
                    xt = pl.tile([P, D], f32, tag="raw")
                    nc.sync.dma_start(out=xt[:], in_=x_dram[c * P : (c + 1) * P, :])
                    stats = pl.tile([P, nsub, 6], f32, tag="stats")
                    for s in range(nsub):
                        nc.vector.bn_stats(
                            out=stats[:, s, :], in_=xt[:, s * fmax : (s + 1) * fmax]
                        )
                    mv = pl.tile([P, 2], f32, tag="mv")
                    nc.vector.bn_aggr(out=mv[:], in_=stats[:])
                    nc.vector.tensor_copy(m_all[:, c : c + 1], mv[:, 0:1])
                    # inv = sqrt(var+eps) = 1/rstd ; rstd = 1/inv
                    nc.scalar.activation(
                        out=inv_all[:, c : c + 1],
                        in_=mv[:, 1:2],
                        func=AF.Sqrt,
                        bias=eps_t[:],
                    )
                    nc.vector.reciprocal(
                        out=rstd_all[:, c : c + 1], in_=inv_all[:, c : c + 1]
                    )
                    # rows: rstd and m*rstd moved into [1, S] via tiny DMA gathers
                    mr = pl.tile([P, 1], f32, tag="mr")
                    nc.vector.tensor_mul(
                        mr[:], m_all[:, c : c + 1], rstd_all[:, c : c + 1]
                    )
                    nc.gpsimd.dma_start(
                        out=rstd_row[:, c * P : (c + 1) * P],
                        in_=rstd_all[:, c : c + 1],
                    )
                    nc.gpsimd.dma_start(out=mr_row[:, c * P : (c + 1) * P], in_=mr[:])
                    # normalized bf16 + transpose to feature-major
                    xn = pl.tile([P, D], bf16, tag="xn")
                    nc.vector.tensor_scalar(
                        out=xn[:],
                        in0=xt[:],
                        scalar1=m_all[:, c : c + 1],
                        scalar2=rstd_all[:, c : c + 1],
                        op0=mybir.AluOpType.subtract,
                        op1=mybir.AluOpType.mult,
                    )
                    for k in range(nk):
                        pt = ptp.tile([P, P], bf16, tag="pt")
                        nc.tensor.transpose(
                            pt[:], xn[:, k * P : (k + 1) * P], ident_bf[:]
                        )
                        nc.scalar.copy(xT[:, k, c * P : (c + 1) * P], pt[:])
            return xT, m_all, rstd_all, inv_all, rstd_row, mr_row

        visT, vis_m, vis_rstd, vis_inv, vis_rsr, vis_mrr = prep(vis, 1024, 1024, "vis")
        txtT, txt_m, txt_rstd, txt_inv, txt_rsr, txt_mrr = prep(txt, 512, 768, "txt")

        # ---------------- gated cross-attention block --------------------
        def gca(p, qnT, q_inv, q_rsr, q_mrr, knT, Sq, Sk, qd, kd, w_dram, att_out_tag):
            """Returns token-major f32 tile [P, Sq//P, E] = qp + gate*att."""
            Kq, Kk, nsq, nsk = qd // P, kd // P, Sq // P, Sk // P
            nnq, nnk = Sq // 512, Sk // 512
            out_att = persist.tile([P, nsq, E], f32, tag=att_out_tag)
            with ExitStack() as ph:
                wp = ph.enter_context(tc.tile_pool(name=p + "_w", bufs=1))
                ap = ph.enter_context(tc.tile_pool(name=p + "_act", bufs=1))
                tp = ph.enter_context(tc.tile_pool(name=p + "_tmp", bufs=3))
                wtp = ph.enter_context(tc.tile_pool(name=p + "_wT", bufs=nsk))
                pmm = ph.enter_context(
                    tc.tile_pool(name=p + "_pmm", bufs=3, space="PSUM")
                )
                psx = ph.enter_context(
                    tc.tile_pool(name=p + "_psx", bufs=2, space="PSUM")
                )
                pcx = ph.enter_context(
                    tc.tile_pool(name=p + "_pcx", bufs=1, space="PSUM")
                )
                pbc = ph.enter_context(
                    tc.tile_pool(name=p + "_pbc", bufs=1, space="PSUM")
                )

                def w_sb(nm, kdim, ncol=E):
                    t = wp.tile([P, kdim, ncol], bf16, tag=nm)
                    nc.sync.dma_start(out=t[:], in_=prm[p + nm][:])
                    return t

                wq = w_sb("_wq", Kq)
                wk = w_sb("_wk", Kk)
                wv = w_sb("_wv", Kk)
                wo = w_sb("_wo", 4)
                wgp = w_sb("_wgp", Kq)
                g1w = w_sb("_g1w", 8)
                g2w = w_sb("_g2w", 4)

                def col_sb(nm, n=4):
                    t = wp.tile([P, n], f32, tag=nm)
                    nc.sync.dma_start(out=t[:], in_=prm[p + nm][:])
                    return t

                bq_c = col_sb("_bq_c")
                bk_c = col_sb("_bk_c")
                bo_c = col_sb("_bo_c")

                def row_sb(nm):
                    t = wp.tile([1, E], f32, tag=nm)
                    nc.sync.dma_start(out=t[:], in_=prm[p + nm][:])
                    return t

                bv_r = row_sb("_bv_r")
                bo_r = row_sb("_bo_r")
                bgp_r = row_sb("_bgp_r")
                cgp_r = row_sb("_cgp_r")
                g1b_r = row_sb("_g1b_r")
                g2b_r = row_sb("_g2b_r")
                gbc = wp.tile([P, E], f32, tag="_gbc")
                nc.sync.dma_start(out=gbc[:], in_=_bcast_ap(bass, prm[p + "_glng"]))
                bbc = wp.tile([P, E], f32, tag="_bbc")
                nc.sync.dma_start(out=bbc[:], in_=_bcast_ap(bass, prm[p + "_glnb"]))

                # --- q^T, k^T feature-major [P, 4, S] bf16 ---
                def proj_T(xT, w, bias_c, K, S, nm):
                    oT = ap.tile([P, 4, S], bf16, tag=nm)
                    for e in range(4):
                        for n in range(S // 512):
                            ps = pmm.tile([P, 512], f32, tag="mm")
                            for k in range(K):
                                nc.tensor.matmul(
                                    ps[:],
                                    lhsT=w[:, k, e * P : (e + 1) * P],
                                    rhs=xT[:, k, n * 512 : (n + 1) * 512],
                                    start=(k == 0),
                                    stop=(k == K - 1),
                                )
                            nc.scalar.activation(
                                out=oT[:, e, n * 512 : (n + 1) * 512],
                                in_=ps[:],
                                func=AF.Identity,
                                bias=bias_c[:, e : e + 1],
                            )
                    return oT

                qT = proj_T(qnT, wq, bq_c, Kq, Sq, "qT")
                kT = proj_T(knT, wk, bk_c, Kk, Sk, "kT")

                # --- v token-major [P, nsk, E] bf16 ---
                v_sb = ap.tile([P, nsk, E], bf16, tag="v")
                for sk in range(nsk):
                    ps = pmm.tile([P, 512], f32, tag="mm")
                    for k in range(Kk):
                        nc.tensor.matmul(
                            ps[:],
                            lhsT=knT[:, k, sk * P : (sk + 1) * P],
                            rhs=wv[:, k, :],
                            start=(k == 0),
                            stop=False,
                        )
                    nc.tensor.matmul(
                        ps[:], lhsT=ones_row[:], rhs=bv_r[:], start=False, stop=True
                    )
                    nc.scalar.copy(v_sb[:, sk, :], ps[:])

                # --- per-head attention ---
                r_all = ap.tile([P, nsq, H], f32, tag="r_all")
                ctxT = ap.tile([P, 4, Sq], bf16, tag="ctxT")
                for h in range(H):
                    th, rh = h // 2, (h % 2) * 64
                    qh = qT[rh : rh + 64, th, :]
                    kh = kT[rh : rh + 64, th, :]
                    # s token-major -> exp/accum -> normalize -> DMA w
                    for sq in range(nsq):
                        wts_n = []
                        acc = tp.tile([P, nnk], f32, tag="acc")
                        for n in range(nnk):
                            ps = psx.tile([P, 512], f32, tag="sx")
                            nc.tensor.matmul(
                                ps[:],
                                lhsT=qT[rh : rh + 64, th, sq * P : (sq + 1) * P],
                                rhs=kT[rh : rh + 64, th, n * 512 : (n + 1) * 512],
                                start=True,
                                stop=True,
                            )
                            wt = tp.tile([P, 512], f32, tag="wtile")
                            wts_n.append(wt)
                            nc.scalar.activation(
                                out=wt[:],
                                in_=ps[:],
                                func=AF.Exp,
                                scale=0.125,
                                accum_out=acc[:, n : n + 1],
                            )
                        rcol = r_all[:, sq, h : h + 1]
                        if nnk == 2:
                            nc.vector.tensor_add(acc[:, 0:1], acc[:, 0:1], acc[:, 1:2])
                        nc.vector.reciprocal(out=rcol, in_=acc[:, 0:1])
                        for n in range(nnk):
                            nc.vector.tensor_scalar_mul(
                                out=wts_n[n][:], in0=wts_n[n][:], scalar1=rcol
                            )
                            nc.sync.dma_start(
                                out=w_dram[
                                    h, sq * P : (sq + 1) * P, n * 512 : (n + 1) * 512
                                ],
                                in_=wts_n[n][:],
                            )
                    # sT -> exp -> bf16 wT tiles
                    wTs = []
                    for sk in range(nsk):
                        wT = wtp.tile([P, Sq], bf16, tag="wT")
                        wTs.append(wT)
                        for n in range(nnq):
                            ps = psx.tile([P, 512], f32, tag="sx")
                            nc.tensor.matmul(
                                ps[:],
                                lhsT=kT[rh : rh + 64, th, sk * P : (sk + 1) * P],
                                rhs=qT[rh : rh + 64, th, n * 512 : (n + 1) * 512],
                                start=True,
                                stop=True,
                            )
                            nc.scalar.activation(
                                out=wT[:, n * 512 : (n + 1) * 512],
                                in_=ps[:],
                                func=AF.Exp,
                                scale=0.125,
                            )
                    # ctx accumulation into pair psum
                    if rh == 0:
                        pc = pcx.tile([P, Sq], f32, tag="ctx")
                    for sk in range(nsk):
                        for n in range(nnq):
                            nc.tensor.matmul(
                                pc[rh : rh + 64, n * 512 : (n + 1) * 512],
                                lhsT=v_sb[:, sk, h * 64 : (h + 1) * 64],
                                rhs=wTs[sk][:, n * 512 : (n + 1) * 512],
                                start=(sk == 0),
                                stop=(sk == nsk - 1),
                            )
                    if rh == 64:
                        # normalize pair: ctxT[:, h//2, :] = pc * bcast(r)
                        rT = ap.tile([2, Sq], f32, tag="rT")
                        for sq in range(nsq):
                            pr = pmm.tile([2, P], f32, tag="mm")
                            nc.tensor.transpose(
                                pr[:], r_all[:, sq, h - 1 : h + 1], ident_f[:]
                            )
                            nc.scalar.copy(rT[:, sq * P : (sq + 1) * P], pr[:])
                        for n in range(nnq):
                            pb = pbc.tile([P, 512], f32, tag="bc")
                            nc.tensor.matmul(
                                pb[:],
                                lhsT=e2[:],
                                rhs=rT[:, n * 512 : (n + 1) * 512],
                                start=True,
                                stop=True,
                            )
                            rbc = tp.tile([P, 512], f32, tag="rbc")
                            nc.scalar.copy(rbc[:], pb[:])
                            nc.vector.tensor_mul(
                                ctxT[:, h // 2, n * 512 : (n + 1) * 512],
                                pc[:, n * 512 : (n + 1) * 512],
                                rbc[:],
                            )

                # --- att feature-major + token-major ---
                attT = ap.tile([P, 4, Sq], bf16, tag="attT")
                for e in range(4):
                    for n in range(nnq):
                        ps = pmm.tile([P, 512], f32, tag="mm")
                        for k in range(4):
                            nc.tensor.matmul(
                                ps[:],
                                lhsT=wo[:, k, e * P : (e + 1) * P],
                                rhs=ctxT[:, k, n * 512 : (n + 1) * 512],
                                start=(k == 0),
                                stop=(k == 3),
                            )
                        nc.scalar.activation(
                            out=attT[:, e, n * 512 : (n + 1) * 512],
                            in_=ps[:],
                            func=AF.Identity,
                            bias=bo_c[:, e : e + 1],
                        )
                att_f = ap.tile([P, nsq, E], bf16, tag="att_f")
                for sq in range(nsq):
                    ps = pmm.tile([P, 512], f32, tag="mm")
                    for k in range(4):
                        nc.tensor.matmul(
                            ps[:],
                            lhsT=ctxT[:, k, sq * P : (sq + 1) * P],
                            rhs=wo[:, k, :],
                            start=(k == 0),
                            stop=False,
                        )
                    nc.tensor.matmul(
                        ps[:], lhsT=ones_row[:], rhs=bo_r[:], start=False, stop=True
                    )
                    nc.scalar.copy(att_f[:, sq, :], ps[:])

                # --- qp token-major bf16 + gate path, fused per sq-chunk ---
                qp_b = ap.tile([P, nsq, E], bf16, tag="qp_b")
                for sq in range(nsq):
                    ps = pmm.tile([P, 512], f32, tag="mm")
                    for k in range(Kq):
                        nc.tensor.matmul(
                            ps[:],
                            lhsT=qnT[:, k, sq * P : (sq + 1) * P],
                            rhs=wgp[:, k, :],
                            start=(k == 0),
                            stop=False,
                        )
                    nc.tensor.matmul(
                        ps[:],
                        lhsT=q_mrr[:, sq * P : (sq + 1) * P],
                        rhs=cgp_r[:],
                        start=False,
                        stop=False,
                    )
                    nc.tensor.matmul(
                        ps[:],
                        lhsT=q_rsr[:, sq * P : (sq + 1) * P],
                        rhs=bgp_r[:],
                        start=False,
                        stop=True,
                    )
                    icol = q_inv[:, sq : sq + 1]
                    nc.scalar.mul(out=qp_b[:, sq, :], in_=ps[:], mul=icol)

                for sq in range(nsq):
                    qpT = tp.tile([P, 4, P], bf16, tag="qpT")
                    for e in range(4):
                        pt = pmm.tile([P, P], bf16, tag="mm")
                        nc.tensor.transpose(
                            pt[:], qp_b[:, sq, e * P : (e + 1) * P], ident_bf[:]
                        )
                        nc.scalar.copy(qpT[:, e, :], pt[:])
                    ps = pmm.tile([P, 512], f32, tag="mm")
                    for k in range(8):
                        if k < 4:
                            lh = qpT[:, k, :]
                        else:
                            lh = attT[:, k - 4, sq * P : (sq + 1) * P]
                        nc.tensor.matmul(
                            ps[:],
                            lhsT=lh,
                            rhs=g1w[:, k, :],
                            start=(k == 0),
                            stop=False,
                        )
                    nc.tensor.matmul(
                        ps[:], lhsT=ones_row[:], rhs=g1b_r[:], start=False, stop=True
                    )
                    stats = tp.tile([P, 6], f32, tag="hstats")
                    nc.vector.bn_stats(out=stats[:], in_=ps[:])
                    mv = tp.tile([P, 2], f32, tag="hmv")
                    nc.vector.bn_aggr(out=mv[:], in_=stats[:])
                    srt = tp.tile([P, 1], f32, tag="hsrt")
                    nc.scalar.activation(
                        out=srt[:], in_=mv[:, 1:2], func=AF.Sqrt, bias=eps_t[:]
                    )
                    rstd2 = tp.tile([P, 1], f32, tag="hrstd")
                    nc.vector.reciprocal(out=rstd2[:], in_=srt[:])
                    hln = tp.tile([P, E], f32, tag="gate")
                    nc.vector.tensor_scalar(
                        out=hln[:],
                        in0=ps[:],
                        scalar1=mv[:, 0:1],
                        scalar2=rstd2[:],
                        op0=mybir.AluOpType.subtract,
                        op1=mybir.AluOpType.mult,
                    )
                    nc.vector.tensor_mul(hln[:], hln[:], gbc[:])
                    nc.vector.tensor_add(hln[:], hln[:], bbc[:])
                    gb = tp.tile([P, E], bf16, tag="gelu")
                    nc.scalar.activation(out=gb[:], in_=hln[:], func=AF.Gelu)
                    gT = tp.tile([P, 4, P], bf16, tag="gTs")
                    for e in range(4):
                        pt = pmm.tile([P, P], bf16, tag="mm")
                        nc.tensor.transpose(
                            pt[:], gb[:, e * P : (e + 1) * P], ident_bf[:]
                        )
                        nc.scalar.copy(gT[:, e, :], pt[:])
                    ps2 = pmm.tile([P, 512], f32, tag="mm")
                    for k in range(4):
                        nc.tensor.matmul(
                            ps2[:],
                            lhsT=gT[:, k, :],
                            rhs=g2w[:, k, :],
                            start=(k == 0),
                            stop=False,
                        )
                    nc.tensor.matmul(
                        ps2[:], lhsT=ones_row[:], rhs=g2b_r[:], start=False, stop=True
                    )
                    gate = tp.tile([P, E], f32, tag="gate")
                    nc.scalar.activation(out=gate[:], in_=ps2[:], func=AF.Sigmoid)
                    nc.vector.tensor_mul(gate[:], gate[:], att_f[:, sq, :])
                    nc.vector.tensor_add(out_att[:, sq, :], gate[:], qp_b[:, sq, :])
            return out_att

        txt_att = gca(
            "t2v", txtT, txt_inv, txt_rsr, txt_mrr, visT,
            512, 1024, 768, 1024, t2v_w, "txt_att",
        )
        vis_att = gca(
            "v2t", visT, vis_inv, vis_rsr, vis_mrr, txtT,
            1024, 512, 1024, 768, v2t_w, "vis_att",
        )

        # ---------------- FFN ----------------
        def ffn(p, x_sb, S, out_dram):
            nsq = S // P
            with ExitStack() as ph:
                wp = ph.enter_context(tc.tile_pool(name=p + "_w", bufs=1))
                tp = ph.enter_context(tc.tile_pool(name=p + "_tmp", bufs=3))
                ap = ph.enter_context(tc.tile_pool(name=p + "_act", bufs=1))
                pmm = ph.enter_context(
                    tc.tile_pool(name=p + "_pmm", bufs=3, space="PSUM")
                )
                w1 = wp.tile([P, 4, 2048], bf16, tag="w1")
                nc.sync.dma_start(out=w1[:], in_=prm[p + "_w1"][:])
                w2 = wp.tile([P, 16, E], bf16, tag="w2")
                nc.sync.dma_start(out=w2[:], in_=prm[p + "_w2"][:])
                b1c = wp.tile([P, 16], f32, tag="b1c")
                nc.sync.dma_start(out=b1c[:], in_=prm[p + "_b1_c"][:])
                b2r = wp.tile([1, E], f32, tag="b2r")
                nc.sync.dma_start(out=b2r[:], in_=prm[p + "_b2_r"][:])

                xlnT = ap.tile([P, 4, S], bf16, tag="xlnT")
                for sq in range(nsq):
                    stats = tp.tile([P, 6], f32, tag="fstats")
                    nc.vector.bn_stats(out=stats[:], in_=x_sb[:, sq, :])
                    mv = tp.tile([P, 2], f32, tag="fmv")
                    nc.vector.bn_aggr(out=mv[:], in_=stats[:])
                    srt = tp.tile([P, 1], f32, tag="fsrt")
                    nc.scalar.activation(
                        out=srt[:], in_=mv[:, 1:2], func=AF.Sqrt, bias=eps_t[:]
                    )
                    rstd = tp.tile([P, 1], f32, tag="frstd")
                    nc.vector.reciprocal(out=rstd[:], in_=srt[:])
                    xln = tp.tile([P, E], bf16, tag="xln")
                    nc.vector.tensor_scalar(
                        out=xln[:],
                        in0=x_sb[:, sq, :],
                        scalar1=mv[:, 0:1],
                        scalar2=rstd[:],
                        op0=mybir.AluOpType.subtract,
                        op1=mybir.AluOpType.mult,
                    )
                    for e in range(4):
                        pt = pmm.tile([P, P], bf16, tag="mm")
                        nc.tensor.transpose(
                            pt[:], xln[:, e * P : (e + 1) * P], ident_bf[:]
                        )
                        nc.scalar.copy(xlnT[:, e, sq * P : (sq + 1) * P], pt[:])
                g1T = ap.tile([P, 16, S], bf16, tag="g1T")
                for m in range(16):
                    for n in range(S // 512):
                        ps = pmm.tile([P, 512], f32, tag="mm")
                        for k in range(4):
                            nc.tensor.matmul(
                                ps[:],
                                lhsT=w1[:, k, m * P : (m + 1) * P],
                                rhs=xlnT[:, k, n * 512 : (n + 1) * 512],
                                start=(k == 0),
                                stop=(k == 3),
                            )
                        nc.scalar.activation(
                            out=g1T[:, m, n * 512 : (n + 1) * 512],
                            in_=ps[:],
                            func=AF.Gelu,
                            bias=b1c[:, m : m + 1],
                        )
                for sq in range(nsq):
                    ps = pmm.tile([P, 512], f32, tag="mm")
                    for k in range(16):
                        nc.tensor.matmul(
                            ps[:],
                            lhsT=g1T[:, k, sq * P : (sq + 1) * P],
                            rhs=w2[:, k, :],
                            start=(k == 0),
                            stop=False,
                        )
                    nc.tensor.matmul(
                        ps[:], lhsT=ones_row[:], rhs=b2r[:], start=False, stop=True
                    )
                    o = tp.tile([P, E], f32, tag="fout")
                    nc.vector.tensor_add(o[:], ps[:], x_sb[:, sq, :])
                    nc.sync.dma_start(
                        out=out_dram[sq * P : (sq + 1) * P, :], in_=o[:]
                    )

        ffn("tff", txt_att, 512, txt_out)
        ffn("vff", vis_att, 1024, vis_out)

    _split_multi_waits(nc, mybir)
    return nc


def _split_multi_waits(nc, mybir):
    """This environment's walrus accepts at most ONE sync wait per
    instruction. Tile emits multi-wait sync_infos; hoist all but the last
    wait onto same-engine NoOp carriers inserted just before."""
    k = 0
    for bb in nc.main_func.blocks:
        lst = bb.instructions
        i = 0
        while i < len(lst):
            inst = lst[i]
            si = inst.sync_info
            if si is not None and len(si.on_wait) > 1:
                extras = list(si.on_wait[:-1])
                for w in extras:
                    nop = mybir.InstNoOp(
                        name=f"zwait-{k}",
                        engine=inst.engine,
                        ins=[],
                        outs=[],
                        sync_info=mybir.SyncInfo(on_wait=[w], on_update=[]),
                    )
                    k += 1
                    lst.insert(i, nop)
                    i += 1
                inst.sync_info = mybir.SyncInfo(
                    on_wait=[si.on_wait[-1]], on_update=list(si.on_update)
                )
            i += 1


def _get_program():
    if "nc" not in _CACHE:
        _CACHE["nc"] = _build_program()
    return _CACHE["nc"]


def kernel(**inputs):
    const = _prep_inputs_host(inputs)
    vis = _f32(inputs["vision_features"])
    txt = _f32(inputs["text_features"])
    B = vis.shape[0]
    in_maps = []
    for b in range(B):
        m = dict(const)
        m["vis"] = np.ascontiguousarray(vis[b])
        m["txt"] = np.ascontiguousarray(txt[b])
        in_maps.append(m)

    nc = _get_program()
    from concourse.bass_utils import run_bass_kernel_spmd

    res = run_bass_kernel_spmd(nc, in_maps, list(range(NCORES))).results

    vision_out = np.stack([res[b]["vision_out"] for b in range(B)])
    text_out = np.stack([res[b]["text_out"] for b in range(B)])
    t2v_w = np.stack([res[b]["t2v_w"] for b in range(B)])
    v2t_w = np.stack([res[b]["v2t_w"] for b in range(B)])
    return (
        vision_out.astype(np.float32),
        text_out.astype(np.float32),
        t2v_w.astype(np.float32),
        v2t_w.astype(np.float32),
    )
